# revision 1
# baseline (speedup 1.0000x reference)
"""Trainium2 Bass kernel for nn_CPAMDec_Mix (dual cross-attention decoder block).

Math per batch sample b (C=512, C4=128, K=64, N=W*H=4096):
    pv1 = wv @ y1^T + bv          [C, K]
    pv2 = wv @ y2^T + bv          [C, K]
    q^T = wq @ x2 + bq            [C4, N]
    kk  = y2 @ wk^T + bk          [K, C4]
    energy = q @ kk^T             [N, K]
    att = softmax(|energy|, -1)   [N, K]
    out1 = scale  * pv1 @ att^T + x1
    out2 = scale1 * pv2 @ att^T + x2

Sharding: pure data parallel — sample b on core b (B == n_cores == 8).
Small weights are replicated; the host pre-transposes them so the kernel
needs no on-chip weight transposes.

Structure: a software pipeline over 4 column-quarters of N. For each
quarter q: attention (q-projection, energy, softmax, transpose) for the
two 512-wide n-tiles of q, then the two output GEMMs + residual for all
four 128-row channel chunks. Attention for q+1 overlaps the output
stores of q and the x1 loads of q+1, keeping DMA and PE dense. Weight /
x2 loads are coalesced into few large DMA instructions so the DMA queue
ramps immediately at kernel start.
"""

import numpy as np

import concourse.bass as bass
import concourse.mybir as mybir
import concourse.tile as tile
from concourse import bacc
from concourse.bass_utils import run_bass_kernel_spmd
from concourse.masks import make_identity

F32 = mybir.dt.float32
F32R = mybir.dt.float32r
AX = mybir.AxisListType
OP = mybir.AluOpType
AF = mybir.ActivationFunctionType

B, C, W, H, K = 8, 512, 64, 64, 64
C4 = C // 4
N = W * H            # 4096
NT = 512             # n-tile (columns per matmul)
NQ = 1024            # quarter width (x2/x1/out DMA chunk)
CC = C // 128        # 4 chunks of 128 over the channel dim

_CACHE = {}


def _attention_quarter(nc, q, x2q, wqT_sb, kkT_sb, bq_sb, ident, pools):
    """Emit attention for the two 512-wide n-tiles of quarter q.
    x2q is the quarter's x2 tile [128, CC*NQ] (chunk cc at cols cc*NQ..).
    Returns the attT tile [K, NQ] (att^T, columns q*NQ..)."""
    psq, pse, pst, qpool, spool, apool = pools
    # attT is consumed by the float32r output matmuls; the scalar-engine
    # copy below rounds it to f32r (TF32-like) as required. Two half
    # tiles per quarter so output matmuls can start mid-attention.
    aTs = []
    for half in range(NQ // NT):
        o = half * NT
        aT = apool.tile([K, NT], F32R, tag="attT")
        aTs.append(aT)
        # q^T tile [C4, NT] = wqT.T @ x2 (+ bq)
        psum_q = psq.tile([C4, NT], F32, tag="psq")
        for cc in range(CC):
            nc.tensor.matmul(
                psum_q[:],
                lhsT=wqT_sb[:, cc * C4 : (cc + 1) * C4],
                rhs=x2q[:, cc * NQ + o : cc * NQ + o + NT],
                start=(cc == 0),
                stop=(cc == CC - 1),
            )
        qT = qpool.tile([C4, NT], F32, tag="qT")
        nc.scalar.activation(qT[:], psum_q[:], AF.Identity, bias=bq_sb[:])

        # energy [n, k] in 128-row chunks: qT_slice.T @ kkT
        psum_e = pse.tile([128, 4 * K], F32, tag="pse")
        for s in range(4):
            nc.tensor.matmul(
                psum_e[:, s * K : (s + 1) * K],
                lhsT=qT[:, s * 128 : (s + 1) * 128],
                rhs=kkT_sb[:],
                start=True,
                stop=True,
            )
        # softmax(|e|) along k (free dim), no max-subtraction:
        # |e| <= ~20 here so exp is safely in fp32 range.
        eexp = spool.tile([128, 4 * K], F32, tag="eexp")
        nc.vector.tensor_scalar(
            eexp[:].bitcast(mybir.dt.uint32),
            psum_e[:].bitcast(mybir.dt.uint32),
            0x7FFFFFFF,
            None,
            op0=OP.bitwise_and,
        )
        nc.scalar.activation(eexp[:], eexp[:], AF.Exp)
        rsum = spool.tile([128, 4], F32, tag="rsum")
        nc.vector.tensor_reduce(
            rsum[:],
            eexp[:].rearrange("p (g d) -> p g d", g=4),
            axis=AX.X,
            op=OP.add,
        )
        rrec = spool.tile([128, 4], F32, tag="rrec")
        nc.vector.reciprocal(rrec[:], rsum[:])
        att = spool.tile([128, 4 * K], F32, tag="att")
        for s in range(4):
            nc.vector.tensor_scalar_mul(
                att[:, s * K : (s + 1) * K],
                eexp[:, s * K : (s + 1) * K],
                rrec[:, s : s + 1],
            )
        # transpose att [n,k] -> attT [k,n]
        psum_t = pst.tile([K, NT], F32, tag="pst")
        for s in range(4):
            nc.tensor.transpose(
                psum_t[:, s * 128 : (s + 1) * 128],
                att[:, s * K : (s + 1) * K],
                ident[:],
            )
        nc.vector.tensor_copy(aT[:], psum_t[:])
    return aTs


def _load_chunked(nc, dst_tile, src_dram, inner):
    """One DMA: [CC*128, inner] DRAM tensor -> [128, CC*inner] SBUF tile
    (row chunk cc lands at columns cc*inner..)."""
    nc.sync.dma_start(
        out=dst_tile[:].rearrange("p (c n) -> p c n", c=CC),
        in_=src_dram[:].rearrange("(c p) n -> p c n", p=128),
    )


def _build_nc():
    nc = bacc.Bacc("TRN2", target_bir_lowering=False, debug=False)

    x1_d = nc.dram_tensor("x1", [C, N], F32, kind="ExternalInput")
    x2_d = nc.dram_tensor("x2", [C, N], F32R, kind="ExternalInput")
    y1T_d = nc.dram_tensor("y1T", [C, K], F32, kind="ExternalInput")
    y2T_d = nc.dram_tensor("y2T", [C, K], F32, kind="ExternalInput")
    wqT_d = nc.dram_tensor("wqT", [C, C4], F32R, kind="ExternalInput")
    wkT_d = nc.dram_tensor("wkT", [C, C4], F32, kind="ExternalInput")
    wvT_d = nc.dram_tensor("wvT", [C, C], F32, kind="ExternalInput")
    # packed per-partition vectors: [bq | bk | scale | scale1]
    vecs_d = nc.dram_tensor("vecs", [C4, 4], F32, kind="ExternalInput")
    # packed rows: [bv (512) | ones (64)]
    rows_d = nc.dram_tensor("rows", [1, C + K], F32, kind="ExternalInput")
    out1_d = nc.dram_tensor("out1", [C, N], F32, kind="ExternalOutput")
    out2_d = nc.dram_tensor("out2", [C, N], F32, kind="ExternalOutput")

    with tile.TileContext(nc) as tc:
        with (
            tc.tile_pool(name="const", bufs=1) as const,
            tc.tile_pool(name="qpool", bufs=3) as qpool,
            tc.tile_pool(name="spool", bufs=3) as spool,
            tc.tile_pool(name="apool", bufs=6) as apool,
            tc.tile_pool(name="x1pool", bufs=10) as x1pool,
            tc.tile_pool(name="o1pool", bufs=3) as o1pool,
            tc.tile_pool(name="o2pool", bufs=3) as o2pool,
            tc.tile_pool(name="psq", bufs=2, space="PSUM") as psq,
            tc.tile_pool(name="pse", bufs=1, space="PSUM") as pse,
            tc.tile_pool(name="pst", bufs=1, space="PSUM") as pst,
            tc.tile_pool(name="pso", bufs=4, space="PSUM") as pso,
        ):
            # ---- weights the attention path needs first (3 DMAs) ----
            wqT_sb = const.tile([128, CC * C4], F32R)
            _load_chunked(nc, wqT_sb, wqT_d, C4)
            wkT_sb = const.tile([128, CC * C4], F32)
            _load_chunked(nc, wkT_sb, wkT_d, C4)
            y2T_sb = const.tile([128, CC * K], F32)
            _load_chunked(nc, y2T_sb, y2T_d, K)
            vecs_sb = const.tile([C4, 4], F32)
            nc.sync.dma_start(out=vecs_sb[:], in_=vecs_d[:])
            bq_sb = vecs_sb[:, 0:1]
            bk_sb = vecs_sb[:, 1:2]
            sc1_sb = vecs_sb[:, 2:3]
            sc2_sb = vecs_sb[:, 3:4]
            rows_sb = const.tile([1, C + K], F32)
            nc.sync.dma_start(out=rows_sb[:], in_=rows_d[:])
            bv_sb = rows_sb[:, 0:C]
            ones_sb = rows_sb[:, C : C + K]
            ident = const.tile([128, 128], F32)
            make_identity(nc, ident[:])

            # ---- x2 quarter 0 (one 2 MB DMA), then the value weights ----
            x2_sb = []
            t = const.tile([128, CC * NQ], F32R, tag="x2_0")
            nc.sync.dma_start(
                out=t[:].rearrange("p (c n) -> p c n", c=CC),
                in_=x2_d[:].rearrange("(c p) n -> p c n", p=128)[:, :, 0:NQ],
            )
            x2_sb.append(t)

            # ---- kk^T (needed by every energy matmul) ----
            pkk = pse.tile([C4, K], F32, tag="pse")
            for cc in range(CC):
                nc.tensor.matmul(
                    pkk[:],
                    lhsT=wkT_sb[:, cc * C4 : (cc + 1) * C4],
                    rhs=y2T_sb[:, cc * K : (cc + 1) * K],
                    start=(cc == 0),
                    stop=(cc == CC - 1),
                )
            kkT_sb = const.tile([C4, K], F32)
            nc.scalar.activation(kkT_sb[:], pkk[:], AF.Identity, bias=bk_sb)

            # ---- value-path weights, rest of x2 ----
            wvT_sb = const.tile([128, CC * C], F32)
            _load_chunked(nc, wvT_sb, wvT_d, C)
            y1T_sb = const.tile([128, CC * K], F32)
            _load_chunked(nc, y1T_sb, y1T_d, K)
            for q in range(1, N // NQ):
                t = const.tile([128, CC * NQ], F32R, tag=f"x2_{q}")
                nc.sync.dma_start(
                    out=t[:].rearrange("p (c n) -> p c n", c=CC),
                    in_=x2_d[:].rearrange("(c p) n -> p c n", p=128)[
                        :, :, q * NQ : (q + 1) * NQ
                    ],
                )
                x2_sb.append(t)

            # ---- pv1^T, pv2^T: [K, C] = y^T.T @ wvT (+ ones^T bv) ----
            pv_sb = []
            for yT_sb in (y1T_sb, y2T_sb):
                ppv = pst.tile([K, C], F32, tag="pst")
                for cc in range(CC):
                    nc.tensor.matmul(
                        ppv[:],
                        lhsT=yT_sb[:, cc * K : (cc + 1) * K],
                        rhs=wvT_sb[:, cc * C : (cc + 1) * C],
                        start=(cc == 0),
                        stop=False,
                    )
                nc.tensor.matmul(
                    ppv[:], lhsT=ones_sb, rhs=bv_sb, start=False, stop=True
                )
                pv = const.tile([K, C], F32R, tag=f"pv_{len(pv_sb)}")
                nc.scalar.copy(pv[:], ppv[:])
                pv_sb.append(pv)
            pv1T_sb, pv2T_sb = pv_sb

            # ---- pipeline over quarters ----
            att_pools = (psq, pse, pst, qpool, spool, apool)
            for q in range(N // NQ):
                aTs = _attention_quarter(
                    nc, q, x2_sb[q], wqT_sb, kkT_sb, bq_sb, ident, att_pools
                )
                for cc in range(CC):
                    x1t = x1pool.tile([128, NQ], F32, tag="x1t")
                    nc.gpsimd.dma_start(
                        out=x1t[:],
                        in_=x1_d[cc * 128 : (cc + 1) * 128, q * NQ : (q + 1) * NQ],
                    )
                    o1 = o1pool.tile([128, NQ], F32, tag="o1")
                    o2 = o2pool.tile([128, NQ], F32, tag="o2")
                    # all four matmuls back-to-back (pso bufs=4) so PE
                    # streams densely; the stt epilogues drain behind.
                    pos = []
                    for pvT in (pv1T_sb, pv2T_sb):
                        for i in range(NQ // NT):
                            po = pso.tile([128, NT], F32, tag="pso")
                            nc.tensor.matmul(
                                po[:],
                                lhsT=pvT[:, cc * 128 : (cc + 1) * 128],
                                rhs=aTs[i][:],
                                start=True,
                                stop=True,
                            )
                            pos.append(po)
                    for j, (sc, ot) in enumerate(((sc1_sb, o1), (sc2_sb, o2))):
                        for i in range(NQ // NT):
                            in1 = (
                                x1t[:, i * NT : (i + 1) * NT]
                                if j == 0
                                else x2_sb[q][:, cc * NQ + i * NT : cc * NQ + (i + 1) * NT].bitcast(F32)
                            )
                            nc.vector.scalar_tensor_tensor(
                                ot[:, i * NT : (i + 1) * NT],
                                in0=pos[j * 2 + i][:],
                                scalar=sc,
                                in1=in1,
                                op0=OP.mult,
                                op1=OP.add,
                            )
                    nc.scalar.dma_start(
                        out=out1_d[cc * 128 : (cc + 1) * 128, q * NQ : (q + 1) * NQ],
                        in_=o1[:],
                    )
                    nc.scalar.dma_start(
                        out=out2_d[cc * 128 : (cc + 1) * 128, q * NQ : (q + 1) * NQ],
                        in_=o2[:],
                    )
    nc.compile()
    return nc


def _get_nc():
    if "nc" not in _CACHE:
        _CACHE["nc"] = _build_nc()
    return _CACHE["nc"]


def kernel(x1, y1, x2, y2, wq, bq, wk, bk, wv, bv, scale, scale1, **run_kwargs):
    x1 = np.asarray(x1, np.float32)
    x2 = np.asarray(x2, np.float32)
    y1 = np.asarray(y1, np.float32)
    y2 = np.asarray(y2, np.float32)
    vecs = np.stack(
        [
            np.asarray(bq, np.float32).reshape(C4),
            np.asarray(bk, np.float32).reshape(C4),
            np.full(C4, np.asarray(scale).reshape(-1)[0], np.float32),
            np.full(C4, np.asarray(scale1).reshape(-1)[0], np.float32),
        ],
        axis=1,
    )
    rows = np.concatenate(
        [np.asarray(bv, np.float32).reshape(C), np.ones(K, np.float32)]
    ).reshape(1, C + K)
    shared = {
        "wqT": np.ascontiguousarray(np.asarray(wq, np.float32).T),
        "wkT": np.ascontiguousarray(np.asarray(wk, np.float32).T),
        "wvT": np.ascontiguousarray(np.asarray(wv, np.float32).T),
        "vecs": np.ascontiguousarray(vecs),
        "rows": rows,
    }
    in_maps = []
    for b in range(B):
        in_maps.append(
            {
                "x1": np.ascontiguousarray(x1[b].reshape(C, N)),
                "x2": np.ascontiguousarray(x2[b].reshape(C, N)),
                "y1T": np.ascontiguousarray(y1[b].T),
                "y2T": np.ascontiguousarray(y2[b].T),
                **shared,
            }
        )
    nc = _get_nc()
    res = run_bass_kernel_spmd(nc, in_maps, list(range(B)), **run_kwargs)
    _CACHE["last_results"] = res
    out1 = np.stack([res.results[b]["out1"].reshape(C, W, H) for b in range(B)])
    out2 = np.stack([res.results[b]["out2"].reshape(C, W, H) for b in range(B)])
    return (out1, out2)



# revision 3
# speedup vs baseline: 1.2416x; 1.2416x over previous
"""Trainium2 Bass kernel for nn_CPAMDec_Mix (dual cross-attention decoder block).

Math per batch sample b (C=512, C4=128, K=64, N=W*H=4096):
    pv1 = wv @ y1^T + bv          [C, K]
    pv2 = wv @ y2^T + bv          [C, K]
    q^T = wq @ x2 + bq            [C4, N]
    kk  = y2 @ wk^T + bk          [K, C4]
    energy = q @ kk^T             [N, K]
    att = softmax(|energy|, -1)   [N, K]
    out1 = scale  * pv1 @ att^T + x1
    out2 = scale1 * pv2 @ att^T + x2

Sharding: pure data parallel — sample b on core b (B == n_cores == 8).

This version runs bf16 end-to-end (correctness gate is l2 < 1e-2; bf16
lands ~2e-3): x1/x2/outputs move over HBM as bf16, halving DMA traffic
vs f32 (16.9 MB/core vs 33.8 MB → ~47 us DMA roofline at 358 GB/s).
The residual adds ride the output matmuls as identity-matmul PSUM
accumulations (scale folded into the pv tiles), so the epilogue is a
single PSUM->SBUF bf16 copy per tile, alternated between the scalar and
vector engines. All x1/x2 loads are queued up front (everything fits in
SBUF at bf16); stores split across both HWDGE queues.
"""

import numpy as np
import ml_dtypes

import concourse.bass as bass
import concourse.mybir as mybir
import concourse.tile as tile
from concourse import bacc
from concourse.bass_utils import run_bass_kernel_spmd
from concourse.masks import make_identity

F32 = mybir.dt.float32
BF16 = mybir.dt.bfloat16
NP_BF16 = np.dtype(ml_dtypes.bfloat16)
AX = mybir.AxisListType
OP = mybir.AluOpType
AF = mybir.ActivationFunctionType

B, C, W, H, K = 8, 512, 64, 64, 64
C4 = C // 4
N = W * H            # 4096
NT = 512             # n-tile (columns per matmul / psum bank)
NQ = 1024            # quarter width (x1/x2/out DMA chunk)
CC = C // 128        # 4 chunks of 128 over the channel dim

_CACHE = {}


def _attention_quarter(nc, x2q, wqT_sb, kkT_sb, bq_sb, ident, pools):
    """Emit attention for the two 512-wide n-tiles of quarter q.
    x2q is the quarter's x2 tile [128, CC*NQ] (chunk cc at cols cc*NQ..).
    Returns the two attT tiles [K, NT] (bf16)."""
    psq, pse, pst, qpool, spool, apool = pools
    aTs = []
    for half in range(NQ // NT):
        o = half * NT
        aT = apool.tile([K, NT], BF16, tag="attT")
        aTs.append(aT)
        # q^T tile [C4, NT] = wqT.T @ x2 (+ bq)
        psum_q = psq.tile([C4, NT], F32, tag="psq")
        for cc in range(CC):
            nc.tensor.matmul(
                psum_q[:],
                lhsT=wqT_sb[:, cc * C4 : (cc + 1) * C4],
                rhs=x2q[:, cc * NQ + o : cc * NQ + o + NT],
                start=(cc == 0),
                stop=(cc == CC - 1),
            )
        qT = qpool.tile([C4, NT], BF16, tag="qT")
        nc.scalar.activation(qT[:], psum_q[:], AF.Identity, bias=bq_sb[:])

        # energy [n, k] in 128-row chunks: qT_slice.T @ kkT
        psum_e = pse.tile([128, 4 * K], F32, tag="pse")
        for s in range(4):
            nc.tensor.matmul(
                psum_e[:, s * K : (s + 1) * K],
                lhsT=qT[:, s * 128 : (s + 1) * 128],
                rhs=kkT_sb[:],
                start=True,
                stop=True,
            )
        # softmax(|e|) along k (free dim), no max-subtraction:
        # |e| <= ~20 here so exp is safely in fp32 range.
        eexp = spool.tile([128, 4 * K], F32, tag="eexp")
        nc.scalar.activation(eexp[:], psum_e[:], AF.Abs)
        nc.scalar.activation(eexp[:], eexp[:], AF.Exp)
        rsum = spool.tile([128, 4], F32, tag="rsum")
        nc.vector.tensor_reduce(
            rsum[:],
            eexp[:].rearrange("p (g d) -> p g d", g=4),
            axis=AX.X,
            op=OP.add,
        )
        rrec = spool.tile([128, 4], F32, tag="rrec")
        nc.vector.reciprocal(rrec[:], rsum[:])
        att = spool.tile([128, 4 * K], BF16, tag="att")
        for s in range(4):
            nc.vector.tensor_scalar_mul(
                att[:, s * K : (s + 1) * K],
                eexp[:, s * K : (s + 1) * K],
                rrec[:, s : s + 1],
            )
        # transpose att [n,k] -> attT [k,n]
        psum_t = pst.tile([K, NT], BF16, tag="pst")
        for s in range(4):
            nc.tensor.transpose(
                psum_t[:, s * 128 : (s + 1) * 128],
                att[:, s * K : (s + 1) * K],
                ident[:],
            )
        nc.vector.tensor_copy(aT[:], psum_t[:])
    return aTs


def _load_chunked(nc, dst_tile, src_dram, inner):
    """One DMA: [CC*128, inner] DRAM tensor -> [128, CC*inner] SBUF tile
    (row chunk cc lands at columns cc*inner..)."""
    nc.sync.dma_start(
        out=dst_tile[:].rearrange("p (c n) -> p c n", c=CC),
        in_=src_dram[:].rearrange("(c p) n -> p c n", p=128),
    )


def _build_nc():
    nc = bacc.Bacc("TRN2", target_bir_lowering=False, debug=False)

    x1_d = nc.dram_tensor("x1", [C, N], BF16, kind="ExternalInput")
    x2_d = nc.dram_tensor("x2", [C, N], BF16, kind="ExternalInput")
    y1T_d = nc.dram_tensor("y1T", [C, K], BF16, kind="ExternalInput")
    y2T_d = nc.dram_tensor("y2T", [C, K], BF16, kind="ExternalInput")
    wqT_d = nc.dram_tensor("wqT", [C, C4], BF16, kind="ExternalInput")
    wkT_d = nc.dram_tensor("wkT", [C, C4], BF16, kind="ExternalInput")
    wvT_d = nc.dram_tensor("wvT", [C, C], BF16, kind="ExternalInput")
    # packed per-partition vectors: [bq | bk | scale | scale1]
    vecs_d = nc.dram_tensor("vecs", [C4, 4], F32, kind="ExternalInput")
    # packed rows: [bv (512) | ones (64)]
    rows_d = nc.dram_tensor("rows", [1, C + K], BF16, kind="ExternalInput")
    out1_d = nc.dram_tensor("out1", [C, N], BF16, kind="ExternalOutput")
    out2_d = nc.dram_tensor("out2", [C, N], BF16, kind="ExternalOutput")

    with tile.TileContext(nc) as tc:
        with (
            tc.tile_pool(name="const", bufs=1) as const,
            tc.tile_pool(name="qpool", bufs=3) as qpool,
            tc.tile_pool(name="spool", bufs=3) as spool,
            tc.tile_pool(name="apool", bufs=6) as apool,
            tc.tile_pool(name="o1pool", bufs=3) as o1pool,
            tc.tile_pool(name="o2pool", bufs=3) as o2pool,
            tc.tile_pool(name="psq", bufs=2, space="PSUM") as psq,
            tc.tile_pool(name="pse", bufs=1, space="PSUM") as pse,
            tc.tile_pool(name="pst", bufs=1, space="PSUM") as pst,
            tc.tile_pool(name="pso", bufs=4, space="PSUM") as pso,
        ):
            # ---- weights the attention path needs first (sync queue) ----
            wqT_sb = const.tile([128, CC * C4], BF16)
            _load_chunked(nc, wqT_sb, wqT_d, C4)
            wkT_sb = const.tile([128, CC * C4], BF16)
            _load_chunked(nc, wkT_sb, wkT_d, C4)
            y2T_sb = const.tile([128, CC * K], BF16)
            _load_chunked(nc, y2T_sb, y2T_d, K)
            vecs_sb = const.tile([C4, 4], F32)
            nc.sync.dma_start(out=vecs_sb[:], in_=vecs_d[:])
            bq_sb = vecs_sb[:, 0:1]
            bk_sb = vecs_sb[:, 1:2]
            sc1_sb = vecs_sb[:, 2:3]
            sc2_sb = vecs_sb[:, 3:4]
            rows_sb = const.tile([1, C + K], BF16)
            nc.sync.dma_start(out=rows_sb[:], in_=rows_d[:])
            bv_sb = rows_sb[:, 0:C]
            ones_sb = rows_sb[:, C : C + K]
            ident = const.tile([128, 128], BF16)
            make_identity(nc, ident[:])

            # ---- x2 quarter 0, value weights, y1 (sync queue) ----
            x2_sb = []
            t = const.tile([128, CC * NQ], BF16, tag="x2_0")
            nc.sync.dma_start(
                out=t[:].rearrange("p (c n) -> p c n", c=CC),
                in_=x2_d[:].rearrange("(c p) n -> p c n", p=128)[:, :, 0:NQ],
            )
            x2_sb.append(t)
            wvT_sb = const.tile([128, CC * C], BF16)
            _load_chunked(nc, wvT_sb, wvT_d, C)
            y1T_sb = const.tile([128, CC * K], BF16)
            _load_chunked(nc, y1T_sb, y1T_d, K)
            for q in range(1, N // NQ):
                t = const.tile([128, CC * NQ], BF16, tag=f"x2_{q}")
                nc.sync.dma_start(
                    out=t[:].rearrange("p (c n) -> p c n", c=CC),
                    in_=x2_d[:].rearrange("(c p) n -> p c n", p=128)[
                        :, :, q * NQ : (q + 1) * NQ
                    ],
                )
                x2_sb.append(t)

            # ---- x1: all four quarters queued up front (gpsimd queue) ----
            x1_sb = []
            for q in range(N // NQ):
                t = const.tile([128, CC * NQ], BF16, tag=f"x1_{q}")
                nc.gpsimd.dma_start(
                    out=t[:].rearrange("p (c n) -> p c n", c=CC),
                    in_=x1_d[:].rearrange("(c p) n -> p c n", p=128)[
                        :, :, q * NQ : (q + 1) * NQ
                    ],
                )
                x1_sb.append(t)

            # ---- kk^T (needed by every energy matmul) ----
            pkk = pse.tile([C4, K], F32, tag="pse")
            for cc in range(CC):
                nc.tensor.matmul(
                    pkk[:],
                    lhsT=wkT_sb[:, cc * C4 : (cc + 1) * C4],
                    rhs=y2T_sb[:, cc * K : (cc + 1) * K],
                    start=(cc == 0),
                    stop=(cc == CC - 1),
                )
            kkT_sb = const.tile([C4, K], BF16)
            nc.scalar.activation(kkT_sb[:], pkk[:], AF.Identity, bias=bk_sb)

            # ---- pv^T tiles [K, C] = scale * (y^T.T @ wvT + ones^T bv) ----
            pv_sb = []
            for yT_sb, sc in ((y1T_sb, sc1_sb), (y2T_sb, sc2_sb)):
                ppv = pst.tile([K, C], F32, tag="pst")
                for cc in range(CC):
                    nc.tensor.matmul(
                        ppv[:],
                        lhsT=yT_sb[:, cc * K : (cc + 1) * K],
                        rhs=wvT_sb[:, cc * C : (cc + 1) * C],
                        start=(cc == 0),
                        stop=False,
                    )
                nc.tensor.matmul(
                    ppv[:], lhsT=ones_sb, rhs=bv_sb, start=False, stop=True
                )
                pv = const.tile([K, C], BF16, tag=f"pv_{len(pv_sb)}")
                nc.scalar.activation(
                    pv[:], ppv[:], AF.Identity, scale=sc[0:K, :]
                )
                pv_sb.append(pv)
            pv1T_sb, pv2T_sb = pv_sb

            # ---- pipeline over quarters ----
            att_pools = (psq, pse, pst, qpool, spool, apool)
            for q in range(N // NQ):
                aTs = _attention_quarter(
                    nc, x2_sb[q], wqT_sb, kkT_sb, bq_sb, ident, att_pools
                )
                for cc in range(CC):
                    o1 = o1pool.tile([128, NQ], BF16, tag="o1")
                    o2 = o2pool.tile([128, NQ], BF16, tag="o2")
                    # psum <- scale*pv@attT, then += I@x (residual), so the
                    # epilogue is a pure psum->sbuf bf16 copy.
                    pos = []
                    for pvT in (pv1T_sb, pv2T_sb):
                        for i in range(NQ // NT):
                            po = pso.tile([128, NT], F32, tag="pso")
                            nc.tensor.matmul(
                                po[:],
                                lhsT=pvT[:, cc * 128 : (cc + 1) * 128],
                                rhs=aTs[i][:],
                                start=True,
                                stop=False,
                            )
                            pos.append(po)
                    for j, xq in ((0, x1_sb[q]), (1, x2_sb[q])):
                        for i in range(NQ // NT):
                            nc.tensor.matmul(
                                pos[j * 2 + i][:],
                                lhsT=ident[:],
                                rhs=xq[:, cc * NQ + i * NT : cc * NQ + (i + 1) * NT],
                                start=False,
                                stop=True,
                            )
                    # drain psum -> sbuf bf16, alternating scalar/vector
                    for j, ot in ((0, o1), (1, o2)):
                        for i in range(NQ // NT):
                            dst = ot[:, i * NT : (i + 1) * NT]
                            src = pos[j * 2 + i][:]
                            if (j + i) % 2 == 0:
                                nc.scalar.activation(dst, src, AF.Identity)
                            else:
                                nc.vector.tensor_copy(dst, src)
                    nc.scalar.dma_start(
                        out=out1_d[cc * 128 : (cc + 1) * 128, q * NQ : (q + 1) * NQ],
                        in_=o1[:],
                    )
                    nc.sync.dma_start(
                        out=out2_d[cc * 128 : (cc + 1) * 128, q * NQ : (q + 1) * NQ],
                        in_=o2[:],
                    )
    nc.compile()
    return nc


def _get_nc():
    if "nc" not in _CACHE:
        _CACHE["nc"] = _build_nc()
    return _CACHE["nc"]


def kernel(x1, y1, x2, y2, wq, bq, wk, bk, wv, bv, scale, scale1, **run_kwargs):
    x1 = np.asarray(x1, np.float32).astype(NP_BF16)
    x2 = np.asarray(x2, np.float32).astype(NP_BF16)
    y1 = np.asarray(y1, np.float32)
    y2 = np.asarray(y2, np.float32)
    vecs = np.stack(
        [
            np.asarray(bq, np.float32).reshape(C4),
            np.asarray(bk, np.float32).reshape(C4),
            np.full(C4, np.asarray(scale).reshape(-1)[0], np.float32),
            np.full(C4, np.asarray(scale1).reshape(-1)[0], np.float32),
        ],
        axis=1,
    )
    rows = np.concatenate(
        [np.asarray(bv, np.float32).reshape(C), np.ones(K, np.float32)]
    ).reshape(1, C + K)
    shared = {
        "wqT": np.ascontiguousarray(np.asarray(wq, np.float32).T).astype(NP_BF16),
        "wkT": np.ascontiguousarray(np.asarray(wk, np.float32).T).astype(NP_BF16),
        "wvT": np.ascontiguousarray(np.asarray(wv, np.float32).T).astype(NP_BF16),
        "vecs": np.ascontiguousarray(vecs),
        "rows": rows.astype(NP_BF16),
    }
    in_maps = []
    for b in range(B):
        in_maps.append(
            {
                "x1": np.ascontiguousarray(x1[b].reshape(C, N)),
                "x2": np.ascontiguousarray(x2[b].reshape(C, N)),
                "y1T": np.ascontiguousarray(y1[b].T).astype(NP_BF16),
                "y2T": np.ascontiguousarray(y2[b].T).astype(NP_BF16),
                **shared,
            }
        )
    nc = _get_nc()
    res = run_bass_kernel_spmd(nc, in_maps, list(range(B)), **run_kwargs)
    _CACHE["last_results"] = res
    out1 = np.stack(
        [res.results[b]["out1"].astype(np.float32).reshape(C, W, H) for b in range(B)]
    )
    out2 = np.stack(
        [res.results[b]["out2"].astype(np.float32).reshape(C, W, H) for b in range(B)]
    )
    return (out1, out2)


# revision 6
# speedup vs baseline: 1.3040x; 1.0502x over previous
"""Trainium2 Bass kernel for nn_CPAMDec_Mix (dual cross-attention decoder block).

Math per batch sample b (C=512, C4=128, K=64, N=W*H=4096):
    pv1 = wv @ y1^T + bv          [C, K]
    pv2 = wv @ y2^T + bv          [C, K]
    q^T = wq @ x2 + bq            [C4, N]
    kk  = y2 @ wk^T + bk          [K, C4]
    energy = q @ kk^T             [N, K]
    att = softmax(|energy|, -1)   [N, K]
    out1 = scale  * pv1 @ att^T + x1
    out2 = scale1 * pv2 @ att^T + x2

Sharding: pure data parallel — sample b on core b (B == n_cores == 8).

bf16 end-to-end (gate is l2 < 1e-2; this lands ~3e-3): x1/x2/outputs move
over HBM as bf16 — 17 MB/core vs 34 MB at f32, i.e. a ~47 us DMA roofline
at 358 GB/s. Per-quarter work is balanced across engines:
  - PE: q-projection, energy, att transposes; the two outputs' pv matmuls
    run as concurrent 64-row tile_position pairs (pv1 rows 0-63 / pv2 rows
    64-127 of one packed lhsT against a row-duplicated attT tile); out2's
    residual rides the PE as an identity-matmul PSUM accumulation.
  - DVE: softmax reductions/normalize + out1's epilogue as one
    tensor_tensor add per tile (psum f32 + resident x1 bf16 -> bf16).
  - ACT: qT bias, |e|, exp, and out2's psum->bf16 copies (residual already
    in psum). Output scales are folded into the pv tiles.
  - Sync engine issues every store (both outputs) plus the small weight /
    x2 quarter-0 loads; gpsimd (SWDGE) streams x1 and x2 quarters 1-3 in
    deadline order. Quarter-0 attention is emitted before the pv matmuls
    so the in-order PE stream isn't stalled behind the wv load.
"""

import numpy as np
import ml_dtypes

import concourse.bass as bass
import concourse.mybir as mybir
import concourse.tile as tile
from concourse import bacc
from concourse.bass_utils import run_bass_kernel_spmd
from concourse.masks import make_identity

F32 = mybir.dt.float32
BF16 = mybir.dt.bfloat16
U32 = mybir.dt.uint32
NP_BF16 = np.dtype(ml_dtypes.bfloat16)
AX = mybir.AxisListType
OP = mybir.AluOpType
AF = mybir.ActivationFunctionType

B, C, W, H, K = 8, 512, 64, 64, 64
C4 = C // 4
N = W * H            # 4096
NT = 512             # n-tile (columns per f32 psum bank)
NQ = 1024            # quarter width (x1/x2/out DMA chunk)
CC = C // 128        # 4 chunks of 128 over the channel dim

_CACHE = {}


def _attention_quarter(nc, x2q, wqT_sb, kkT_sb, bq_sb, ident, pools):
    """Emit attention for one quarter. Returns aTd [128, NQ] bf16 with
    attT in rows 0-63 and a duplicate in rows 64-127 (so the two outputs'
    pv matmuls can run as concurrent 64-row tile_position pairs)."""
    psq, pse, pst, qpool, spool, apool = pools
    aTd = apool.tile([128, NQ], BF16, tag="attT")
    psum_t = pst.tile([K, NQ], BF16, tag="pst")
    for half in range(NQ // NT):
        o = half * NT
        # q^T tile [C4, NT] = wqT.T @ x2 (+ bq)
        psum_q = psq.tile([C4, NT], F32, tag="psq")
        for cc in range(CC):
            nc.tensor.matmul(
                psum_q[:],
                lhsT=wqT_sb[:, cc * C4 : (cc + 1) * C4],
                rhs=x2q[:, cc * NQ + o : cc * NQ + o + NT],
                start=(cc == 0),
                stop=(cc == CC - 1),
            )
        qT = qpool.tile([C4, NT], BF16, tag="qT")
        nc.scalar.activation(qT[:], psum_q[:], AF.Identity, bias=bq_sb[:])

        # energy [n, k] in 128-row chunks: qT_slice.T @ kkT
        psum_e = pse.tile([128, 4 * K], F32, tag="pse")
        for s in range(4):
            nc.tensor.matmul(
                psum_e[:, s * K : (s + 1) * K],
                lhsT=qT[:, s * 128 : (s + 1) * 128],
                rhs=kkT_sb[:],
                start=True,
                stop=True,
            )
        # softmax(|e|) along k (free dim), no max-subtraction:
        # |e| <= ~20 here so exp is safely in fp32 range.
        eexp = spool.tile([128, 4 * K], F32, tag="eexp")
        nc.scalar.activation(eexp[:], psum_e[:], AF.Abs)
        nc.scalar.activation(eexp[:], eexp[:], AF.Exp)
        rsum = spool.tile([128, 4], F32, tag="rsum")
        nc.vector.tensor_reduce(
            rsum[:],
            eexp[:].rearrange("p (g d) -> p g d", g=4),
            axis=AX.X,
            op=OP.add,
        )
        rrec = spool.tile([128, 4], F32, tag="rrec")
        nc.vector.reciprocal(rrec[:], rsum[:])
        att = spool.tile([128, 4 * K], BF16, tag="att")
        for s in range(4):
            nc.vector.tensor_scalar_mul(
                att[:, s * K : (s + 1) * K],
                eexp[:, s * K : (s + 1) * K],
                rrec[:, s : s + 1],
            )
        # transpose att [n,k] -> attT [k,n] into the shared bf16 psum bank
        for s in range(4):
            nc.tensor.transpose(
                psum_t[:, o + s * 128 : o + (s + 1) * 128],
                att[:, s * K : (s + 1) * K],
                ident[:],
            )
    # move attT to SBUF twice (rows 0-63 and 64-127), bf16 pairs as u32
    nc.vector.tensor_copy(
        aTd[0:K, :].bitcast(U32), psum_t[:].bitcast(U32)
    )
    nc.vector.tensor_copy(
        aTd[K : 2 * K, :].bitcast(U32), psum_t[:].bitcast(U32)
    )
    return aTd


def _load_chunked(nc, dst_tile, src_dram, inner):
    """One DMA: [CC*128, inner] DRAM tensor -> [128, CC*inner] SBUF tile
    (row chunk cc lands at columns cc*inner..)."""
    nc.sync.dma_start(
        out=dst_tile[:].rearrange("p (c n) -> p c n", c=CC),
        in_=src_dram[:].rearrange("(c p) n -> p c n", p=128),
    )


def _build_nc():
    nc = bacc.Bacc("TRN2", target_bir_lowering=False, debug=False)

    x1_d = nc.dram_tensor("x1", [C, N], BF16, kind="ExternalInput")
    x2_d = nc.dram_tensor("x2", [C, N], BF16, kind="ExternalInput")
    y1T_d = nc.dram_tensor("y1T", [C, K], BF16, kind="ExternalInput")
    y2T_d = nc.dram_tensor("y2T", [C, K], BF16, kind="ExternalInput")
    wqT_d = nc.dram_tensor("wqT", [C, C4], BF16, kind="ExternalInput")
    wkT_d = nc.dram_tensor("wkT", [C, C4], BF16, kind="ExternalInput")
    wvT_d = nc.dram_tensor("wvT", [C, C], BF16, kind="ExternalInput")
    # packed per-partition vectors: [bq | bk | scale | scale1]
    vecs_d = nc.dram_tensor("vecs", [C4, 4], F32, kind="ExternalInput")
    # packed rows: [bv (512) | ones (64)]
    rows_d = nc.dram_tensor("rows", [1, C + K], BF16, kind="ExternalInput")
    out1_d = nc.dram_tensor("out1", [C, N], BF16, kind="ExternalOutput")
    out2_d = nc.dram_tensor("out2", [C, N], BF16, kind="ExternalOutput")

    with tile.TileContext(nc) as tc:
        with (
            tc.tile_pool(name="const", bufs=1) as const,
            tc.tile_pool(name="qpool", bufs=3) as qpool,
            tc.tile_pool(name="spool", bufs=3) as spool,
            tc.tile_pool(name="apool", bufs=3) as apool,
            tc.tile_pool(name="o1pool", bufs=3) as o1pool,
            tc.tile_pool(name="o2pool", bufs=3) as o2pool,
            tc.tile_pool(name="psq", bufs=2, space="PSUM") as psq,
            tc.tile_pool(name="pse", bufs=1, space="PSUM") as pse,
            tc.tile_pool(name="pst", bufs=1, space="PSUM") as pst,
            tc.tile_pool(name="pso", bufs=4, space="PSUM") as pso,
        ):
            # ---- weights the attention path needs first (sync queue) ----
            wqT_sb = const.tile([128, CC * C4], BF16)
            _load_chunked(nc, wqT_sb, wqT_d, C4)
            wkT_sb = const.tile([128, CC * C4], BF16)
            _load_chunked(nc, wkT_sb, wkT_d, C4)
            y2T_sb = const.tile([128, CC * K], BF16)
            _load_chunked(nc, y2T_sb, y2T_d, K)
            vecs_sb = const.tile([C4, 4], F32)
            nc.sync.dma_start(out=vecs_sb[:], in_=vecs_d[:])
            bq_sb = vecs_sb[:, 0:1]
            bk_sb = vecs_sb[:, 1:2]
            sc1_sb = vecs_sb[:, 2:3]
            sc2_sb = vecs_sb[:, 3:4]
            rows_sb = const.tile([1, C + K], BF16)
            nc.sync.dma_start(out=rows_sb[:], in_=rows_d[:])
            bv_sb = rows_sb[:, 0:C]
            ones_sb = rows_sb[:, C : C + K]
            ident = const.tile([128, 128], BF16)
            make_identity(nc, ident[:])

            # ---- x2 quarter 0, value weights (sync queue) ----
            x2_sb = [None] * (N // NQ)
            t0 = const.tile([128, CC * NQ], BF16, tag="x2_0")
            nc.sync.dma_start(
                out=t0[:].rearrange("p (c n) -> p c n", c=CC),
                in_=x2_d[:].rearrange("(c p) n -> p c n", p=128)[:, :, 0:NQ],
            )
            x2_sb[0] = t0
            wvT_sb = const.tile([128, CC * C], BF16)
            _load_chunked(nc, wvT_sb, wvT_d, C)
            y1T_sb = const.tile([128, CC * K], BF16)
            _load_chunked(nc, y1T_sb, y1T_d, K)

            # ---- x1 + x2[1:] on the gpsimd (SWDGE) queue, deadline order:
            # x1_0(out q0), x2_1(att q1), x1_1, x2_2, x1_2, x2_3, x1_3 ----
            x1_sb = [None] * (N // NQ)

            def _load_quarter(dram, q, tag):
                t = const.tile([128, CC * NQ], BF16, tag=tag)
                nc.gpsimd.dma_start(
                    out=t[:].rearrange("p (c n) -> p c n", c=CC),
                    in_=dram[:].rearrange("(c p) n -> p c n", p=128)[
                        :, :, q * NQ : (q + 1) * NQ
                    ],
                )
                return t

            x1_sb[0] = _load_quarter(x1_d, 0, "x1_0")
            for q in range(1, N // NQ):
                x2_sb[q] = _load_quarter(x2_d, q, f"x2_{q}")
                x1_sb[q] = _load_quarter(x1_d, q, f"x1_{q}")

            # ---- kk^T (needed by every energy matmul) ----
            pkk = pse.tile([C4, K], F32, tag="pse")
            for cc in range(CC):
                nc.tensor.matmul(
                    pkk[:],
                    lhsT=wkT_sb[:, cc * C4 : (cc + 1) * C4],
                    rhs=y2T_sb[:, cc * K : (cc + 1) * K],
                    start=(cc == 0),
                    stop=(cc == CC - 1),
                )
            kkT_sb = const.tile([C4, K], BF16)
            nc.scalar.activation(kkT_sb[:], pkk[:], AF.Identity, bias=bk_sb)

            att_pools = (psq, pse, pst, qpool, spool, apool)
            # attention for quarter 0 first: it only needs wq/wk/y2/x2_0,
            # so the in-order PE stream isn't stalled behind the wv load.
            aTd0 = _attention_quarter(
                nc, x2_sb[0], wqT_sb, kkT_sb, bq_sb, ident, att_pools
            )

            # ---- packed pv tile [128, C] bf16: rows 0-63 scale*pv1T,
            # rows 64-127 scale1*pv2T ----
            pv12_sb = const.tile([128, C], BF16)
            for idx, (yT_sb, sc) in enumerate(((y1T_sb, sc1_sb), (y2T_sb, sc2_sb))):
                ppv = pst.tile([K, C], F32, tag="pst")
                for cc in range(CC):
                    nc.tensor.matmul(
                        ppv[:],
                        lhsT=yT_sb[:, cc * K : (cc + 1) * K],
                        rhs=wvT_sb[:, cc * C : (cc + 1) * C],
                        start=(cc == 0),
                        stop=False,
                    )
                nc.tensor.matmul(
                    ppv[:], lhsT=ones_sb, rhs=bv_sb, start=False, stop=True
                )
                nc.scalar.activation(
                    pv12_sb[idx * K : (idx + 1) * K, :],
                    ppv[:],
                    AF.Identity,
                    scale=sc[0:K, :],
                )

            # ---- pipeline over quarters ----
            aTd = aTd0
            for q in range(N // NQ):
                for cc in range(CC):
                    o1 = o1pool.tile([128, NQ], BF16, tag="o1")
                    o2 = o2pool.tile([128, NQ], BF16, tag="o2")
                    pv1c = pv12_sb[0:K, cc * 128 : (cc + 1) * 128]
                    pv2c = pv12_sb[K : 2 * K, cc * 128 : (cc + 1) * 128]
                    pos = []
                    for i in range(NQ // NT):
                        nt = slice(i * NT, (i + 1) * NT)
                        po1 = pso.tile([128, NT], F32, tag="pso")
                        po2 = pso.tile([128, NT], F32, tag="pso")
                        # concurrent 64-row tile pair (rows 0-63 / 64-127)
                        nc.tensor.matmul(
                            po1[:], lhsT=pv1c, rhs=aTd[0:K, nt],
                            start=True, stop=True,
                        )
                        nc.tensor.matmul(
                            po2[:], lhsT=pv2c, rhs=aTd[K : 2 * K, nt],
                            start=True, stop=False,
                        )
                        pos.append((po1, po2))
                    # out2 residual rides the PE (identity accumulate)
                    for i in range(NQ // NT):
                        nc.tensor.matmul(
                            pos[i][1][:],
                            lhsT=ident[:],
                            rhs=x2_sb[q][:, cc * NQ + i * NT : cc * NQ + (i + 1) * NT],
                            start=False,
                            stop=True,
                        )
                    # out1: single DVE op = evacuate + residual
                    for i in range(NQ // NT):
                        nt = slice(i * NT, (i + 1) * NT)
                        nc.vector.tensor_tensor(
                            o1[:, nt],
                            pos[i][0][:],
                            x1_sb[q][:, cc * NQ + i * NT : cc * NQ + (i + 1) * NT],
                            op=OP.add,
                        )
                        nc.scalar.activation(o2[:, nt], pos[i][1][:], AF.Identity)
                    nc.sync.dma_start(
                        out=out1_d[cc * 128 : (cc + 1) * 128, q * NQ : (q + 1) * NQ],
                        in_=o1[:],
                    )
                    nc.sync.dma_start(
                        out=out2_d[cc * 128 : (cc + 1) * 128, q * NQ : (q + 1) * NQ],
                        in_=o2[:],
                    )
                if q + 1 < N // NQ:
                    aTd = _attention_quarter(
                        nc, x2_sb[q + 1], wqT_sb, kkT_sb, bq_sb, ident, att_pools
                    )
    nc.compile()
    return nc


def _get_nc():
    if "nc" not in _CACHE:
        _CACHE["nc"] = _build_nc()
    return _CACHE["nc"]


def kernel(x1, y1, x2, y2, wq, bq, wk, bk, wv, bv, scale, scale1, **run_kwargs):
    x1 = np.asarray(x1, np.float32).astype(NP_BF16)
    x2 = np.asarray(x2, np.float32).astype(NP_BF16)
    y1 = np.asarray(y1, np.float32)
    y2 = np.asarray(y2, np.float32)
    vecs = np.stack(
        [
            np.asarray(bq, np.float32).reshape(C4),
            np.asarray(bk, np.float32).reshape(C4),
            np.full(C4, np.asarray(scale).reshape(-1)[0], np.float32),
            np.full(C4, np.asarray(scale1).reshape(-1)[0], np.float32),
        ],
        axis=1,
    )
    rows = np.concatenate(
        [np.asarray(bv, np.float32).reshape(C), np.ones(K, np.float32)]
    ).reshape(1, C + K)
    shared = {
        "wqT": np.ascontiguousarray(np.asarray(wq, np.float32).T).astype(NP_BF16),
        "wkT": np.ascontiguousarray(np.asarray(wk, np.float32).T).astype(NP_BF16),
        "wvT": np.ascontiguousarray(np.asarray(wv, np.float32).T).astype(NP_BF16),
        "vecs": np.ascontiguousarray(vecs),
        "rows": rows.astype(NP_BF16),
    }
    in_maps = []
    for b in range(B):
        in_maps.append(
            {
                "x1": np.ascontiguousarray(x1[b].reshape(C, N)),
                "x2": np.ascontiguousarray(x2[b].reshape(C, N)),
                "y1T": np.ascontiguousarray(y1[b].T).astype(NP_BF16),
                "y2T": np.ascontiguousarray(y2[b].T).astype(NP_BF16),
                **shared,
            }
        )
    nc = _get_nc()
    res = run_bass_kernel_spmd(nc, in_maps, list(range(B)), **run_kwargs)
    _CACHE["last_results"] = res
    out1 = np.stack(
        [res.results[b]["out1"].astype(np.float32).reshape(C, W, H) for b in range(B)]
    )
    out2 = np.stack(
        [res.results[b]["out2"].astype(np.float32).reshape(C, W, H) for b in range(B)]
    )
    return (out1, out2)


# revision 8
# speedup vs baseline: 1.5282x; 1.1720x over previous
"""Trainium2 Bass kernel for nn_CPAMDec_Mix (dual cross-attention decoder block).

Math per batch sample b (C=512, C4=128, K=64, N=W*H=4096):
    pv1 = wv @ y1^T + bv          [C, K]
    pv2 = wv @ y2^T + bv          [C, K]
    q^T = wq @ x2 + bq            [C4, N]
    kk  = y2 @ wk^T + bk          [K, C4]
    energy = q @ kk^T             [N, K]
    att = softmax(|energy|, -1)   [N, K]
    out1 = scale  * pv1 @ att^T + x1
    out2 = scale1 * pv2 @ att^T + x2

Sharding: pure data parallel — sample b on core b (B == n_cores == 8).

bf16 end-to-end (gate l2 < 1e-2; this lands ~3e-3): 17 MB/core of HBM
traffic vs 34 MB at f32 -> ~47 us DMA roofline at 358 GB/s. The kernel is
latency-limited, so everything is shaped to minimize per-op overhead and
cross-engine handoffs:
  - full-quarter (1024-col) granularity: one qT activation, one abs, one
    exp, one reduce, one normalize, one attT evacuation per quarter; out
    tiles use 2-bank [128, 1024] f32 PSUM slots so each (chunk, output)
    evacuates in a single wide op.
  - software-pipelined emission: quarter q+1's attention stages are woven
    between quarter q's four out-chunks, so the in-order PE stream always
    has runnable matmuls and HAM stays un-throttled.
  - epilogues: out1 = one DVE tensor_tensor (psum f32 + x1 bf16 -> bf16,
    residual + evacuation fused); out2's residual rides the PE as an
    identity-matmul accumulation, evacuated by one scalar-engine copy.
    Output scales are folded into the pv tiles.
  - sync engine issues all stores + the small weight / x2 quarter-0
    loads; gpsimd (SWDGE) streams x1 and x2 quarters 1-3 deadline-ordered.
"""

import numpy as np
import ml_dtypes

import concourse.bass as bass
import concourse.mybir as mybir
import concourse.tile as tile
from concourse import bacc
from concourse.bass_utils import run_bass_kernel_spmd
from concourse.masks import make_identity

F32 = mybir.dt.float32
BF16 = mybir.dt.bfloat16
U32 = mybir.dt.uint32
NP_BF16 = np.dtype(ml_dtypes.bfloat16)
AX = mybir.AxisListType
OP = mybir.AluOpType
AF = mybir.ActivationFunctionType

B, C, W, H, K = 8, 512, 64, 64, 64
C4 = C // 4
N = W * H            # 4096
NT = 512             # columns per f32 psum bank / matmul
NQ = 1024            # quarter width
CC = C // 128        # 4 chunks of 128 over the channel dim
NHALF = NQ // NT     # 2

_CACHE = {}


class _AttQuarter:
    """Attention for one quarter, split into 4 emission stages so the
    caller can weave them between the previous quarter's out-chunks."""

    def __init__(self, nc, x2q, ctx):
        self.nc = nc
        self.x2q = x2q
        self.ctx = ctx

    def stage0(self):  # q-projection
        nc, c = self.nc, self.ctx
        self.psum_q = c["psq"].tile([C4, NQ], F32, tag="psq")
        for half in range(NHALF):
            o = half * NT
            for cc in range(CC):
                nc.tensor.matmul(
                    self.psum_q[:, o : o + NT],
                    lhsT=c["wqT"][:, cc * C4 : (cc + 1) * C4],
                    rhs=self.x2q[:, cc * NQ + o : cc * NQ + o + NT],
                    start=(cc == 0),
                    stop=(cc == CC - 1),
                )
        self.qT = c["qpool"].tile([C4, NQ], BF16, tag="qT")
        nc.scalar.activation(self.qT[:], self.psum_q[:], AF.Identity, bias=c["bq"])

    def stage1(self):  # energy + |e| + exp
        nc, c = self.nc, self.ctx
        self.psum_e = c["ept"].tile([128, 8 * K], F32, tag="ept")
        for s in range(8):
            nc.tensor.matmul(
                self.psum_e[:, s * K : (s + 1) * K],
                lhsT=self.qT[:, s * 128 : (s + 1) * 128],
                rhs=c["kkT"],
                start=True,
                stop=True,
            )
        self.eabs = c["spool"].tile([128, 8 * K], F32, tag="eabs")
        nc.vector.tensor_scalar(
            self.eabs[:].bitcast(U32),
            self.psum_e[:].bitcast(U32),
            0x7FFFFFFF,
            None,
            op0=OP.bitwise_and,
        )
        self.eexp = c["spool"].tile([128, 8 * K], BF16, tag="eexp")
        nc.scalar.activation(self.eexp[:], self.eabs[:], AF.Exp)

    def stage2(self):  # softmax normalize + transpose
        nc, c = self.nc, self.ctx
        rsum = c["spool"].tile([128, 8], F32, tag="rsum")
        nc.vector.tensor_reduce(
            rsum[:],
            self.eexp[:].rearrange("p (g d) -> p g d", g=8),
            axis=AX.X,
            op=OP.add,
        )
        rrec = c["spool"].tile([128, 8], F32, tag="rrec")
        nc.vector.reciprocal(rrec[:], rsum[:])
        att = c["spool"].tile([128, 8 * K], BF16, tag="att")
        nc.vector.tensor_tensor(
            att[:].rearrange("p (g d) -> p g d", g=8),
            self.eexp[:].rearrange("p (g d) -> p g d", g=8),
            rrec[:].unsqueeze(2).broadcast_to((128, 8, K)),
            op=OP.mult,
        )
        self.psum_t = c["ept"].tile([K, NQ], BF16, tag="ept")
        for s in range(8):
            nc.tensor.transpose(
                self.psum_t[:, s * 128 : (s + 1) * 128],
                att[:, s * K : (s + 1) * K],
                c["ident"],
            )

    def stage3(self):  # attT -> SBUF
        nc, c = self.nc, self.ctx
        self.aT = c["apool"].tile([K, NQ], BF16, tag="attT")
        nc.vector.tensor_copy(
            self.aT[:].bitcast(U32), self.psum_t[:].bitcast(U32)
        )
        return self.aT

    def run_all(self):
        self.stage0()
        self.stage1()
        self.stage2()
        return self.stage3()


def _load_chunked(nc, dst_tile, src_dram, inner):
    """One DMA: [CC*128, inner] DRAM tensor -> [128, CC*inner] SBUF tile
    (row chunk cc lands at columns cc*inner..)."""
    nc.sync.dma_start(
        out=dst_tile[:].rearrange("p (c n) -> p c n", c=CC),
        in_=src_dram[:].rearrange("(c p) n -> p c n", p=128),
    )


def _build_nc():
    nc = bacc.Bacc("TRN2", target_bir_lowering=False, debug=False)

    x1_d = nc.dram_tensor("x1", [C, N], BF16, kind="ExternalInput")
    x2_d = nc.dram_tensor("x2", [C, N], BF16, kind="ExternalInput")
    y1T_d = nc.dram_tensor("y1T", [C, K], BF16, kind="ExternalInput")
    y2T_d = nc.dram_tensor("y2T", [C, K], BF16, kind="ExternalInput")
    wqT_d = nc.dram_tensor("wqT", [C, C4], BF16, kind="ExternalInput")
    wkT_d = nc.dram_tensor("wkT", [C, C4], BF16, kind="ExternalInput")
    wvT_d = nc.dram_tensor("wvT", [C, C], BF16, kind="ExternalInput")
    # packed per-partition vectors: [bq | bk | scale | scale1]
    vecs_d = nc.dram_tensor("vecs", [C4, 4], F32, kind="ExternalInput")
    # packed rows: [bv (512) | ones (64)]
    rows_d = nc.dram_tensor("rows", [1, C + K], BF16, kind="ExternalInput")
    out1_d = nc.dram_tensor("out1", [C, N], BF16, kind="ExternalOutput")
    out2_d = nc.dram_tensor("out2", [C, N], BF16, kind="ExternalOutput")

    with tile.TileContext(nc) as tc:
        with (
            tc.tile_pool(name="const", bufs=1) as const,
            tc.tile_pool(name="qpool", bufs=2) as qpool,
            tc.tile_pool(name="spool", bufs=2) as spool,
            tc.tile_pool(name="apool", bufs=2) as apool,
            tc.tile_pool(name="o1pool", bufs=3) as o1pool,
            tc.tile_pool(name="o2pool", bufs=3) as o2pool,
            tc.tile_pool(name="psq", bufs=1, space="PSUM") as psq,
            tc.tile_pool(name="ept", bufs=2, space="PSUM") as ept,
            tc.tile_pool(name="pso", bufs=2, space="PSUM") as pso,
        ):
            # ---- weights the attention path needs first (sync queue) ----
            wqT_sb = const.tile([128, CC * C4], BF16)
            _load_chunked(nc, wqT_sb, wqT_d, C4)
            wkT_sb = const.tile([128, CC * C4], BF16)
            _load_chunked(nc, wkT_sb, wkT_d, C4)
            y2T_sb = const.tile([128, CC * K], BF16)
            _load_chunked(nc, y2T_sb, y2T_d, K)
            vecs_sb = const.tile([C4, 4], F32)
            nc.sync.dma_start(out=vecs_sb[:], in_=vecs_d[:])
            bq_sb = vecs_sb[:, 0:1]
            bk_sb = vecs_sb[:, 1:2]
            sc1_sb = vecs_sb[:, 2:3]
            sc2_sb = vecs_sb[:, 3:4]
            rows_sb = const.tile([1, C + K], BF16)
            nc.sync.dma_start(out=rows_sb[:], in_=rows_d[:])
            bv_sb = rows_sb[:, 0:C]
            ones_sb = rows_sb[:, C : C + K]
            ident = const.tile([128, 128], BF16)
            make_identity(nc, ident[:])

            # ---- x2 quarter 0, value weights (sync queue) ----
            x2_sb = [None] * (N // NQ)
            t0 = const.tile([128, CC * NQ], BF16, tag="x2_0")
            nc.sync.dma_start(
                out=t0[:].rearrange("p (c n) -> p c n", c=CC),
                in_=x2_d[:].rearrange("(c p) n -> p c n", p=128)[:, :, 0:NQ],
            )
            x2_sb[0] = t0
            wvT_sb = const.tile([128, CC * C], BF16)
            _load_chunked(nc, wvT_sb, wvT_d, C)
            y1T_sb = const.tile([128, CC * K], BF16)
            _load_chunked(nc, y1T_sb, y1T_d, K)

            # ---- x1 + x2[1:] on the gpsimd (SWDGE) queue, deadline order ----
            x1_sb = [None] * (N // NQ)

            def _load_quarter(dram, q, tag):
                t = const.tile([128, CC * NQ], BF16, tag=tag)
                nc.gpsimd.dma_start(
                    out=t[:].rearrange("p (c n) -> p c n", c=CC),
                    in_=dram[:].rearrange("(c p) n -> p c n", p=128)[
                        :, :, q * NQ : (q + 1) * NQ
                    ],
                )
                return t

            x1_sb[0] = _load_quarter(x1_d, 0, "x1_0")
            for q in range(1, N // NQ):
                x2_sb[q] = _load_quarter(x2_d, q, f"x2_{q}")
                x1_sb[q] = _load_quarter(x1_d, q, f"x1_{q}")

            # ---- kk^T (needed by every energy matmul) ----
            pkk = ept.tile([C4, K], F32, tag="ept")
            for cc in range(CC):
                nc.tensor.matmul(
                    pkk[:],
                    lhsT=wkT_sb[:, cc * C4 : (cc + 1) * C4],
                    rhs=y2T_sb[:, cc * K : (cc + 1) * K],
                    start=(cc == 0),
                    stop=(cc == CC - 1),
                )
            kkT_sb = const.tile([C4, K], BF16)
            nc.scalar.activation(kkT_sb[:], pkk[:], AF.Identity, bias=bk_sb)

            ctx = {
                "psq": psq, "ept": ept, "qpool": qpool, "spool": spool,
                "apool": apool, "wqT": wqT_sb[:], "kkT": kkT_sb[:],
                "bq": bq_sb, "ident": ident[:],
            }

            # attention for quarter 0 up front (only needs wq/wk/y2/x2_0,
            # so the in-order PE stream isn't stalled behind the wv load)
            aT = _AttQuarter(nc, x2_sb[0], ctx).run_all()

            # ---- pv^T tiles [K, C] = scale * (y^T.T @ wvT + ones^T bv) ----
            pv_sb = []
            for yT_sb, sc in ((y1T_sb, sc1_sb), (y2T_sb, sc2_sb)):
                ppv = ept.tile([K, C], F32, tag="ept")
                for cc in range(CC):
                    nc.tensor.matmul(
                        ppv[:],
                        lhsT=yT_sb[:, cc * K : (cc + 1) * K],
                        rhs=wvT_sb[:, cc * C : (cc + 1) * C],
                        start=(cc == 0),
                        stop=False,
                    )
                nc.tensor.matmul(
                    ppv[:], lhsT=ones_sb, rhs=bv_sb, start=False, stop=True
                )
                pv = const.tile([K, C], BF16, tag=f"pv_{len(pv_sb)}")
                nc.scalar.activation(pv[:], ppv[:], AF.Identity, scale=sc[0:K, :])
                pv_sb.append(pv)
            pv1T_sb, pv2T_sb = pv_sb

            # ---- software-pipelined quarters: out(q) woven with att(q+1) ----
            for q in range(N // NQ):
                nxt = (
                    _AttQuarter(nc, x2_sb[q + 1], ctx)
                    if q + 1 < N // NQ
                    else None
                )
                for cc in range(CC):
                    po1 = pso.tile([128, NQ], F32, tag="po")
                    po2 = pso.tile([128, NQ], F32, tag="po")
                    pv1c = pv1T_sb[:, cc * 128 : (cc + 1) * 128]
                    pv2c = pv2T_sb[:, cc * 128 : (cc + 1) * 128]
                    for i in range(NHALF):
                        nt = slice(i * NT, (i + 1) * NT)
                        nc.tensor.matmul(
                            po1[:, nt], lhsT=pv1c, rhs=aT[:, nt],
                            start=True, stop=True,
                        )
                    for i in range(NHALF):
                        nt = slice(i * NT, (i + 1) * NT)
                        nc.tensor.matmul(
                            po2[:, nt], lhsT=pv2c, rhs=aT[:, nt],
                            start=True, stop=False,
                        )
                    for i in range(NHALF):
                        nt = slice(i * NT, (i + 1) * NT)
                        nc.tensor.matmul(
                            po2[:, nt],
                            lhsT=ident[:],
                            rhs=x2_sb[q][:, cc * NQ + i * NT : cc * NQ + (i + 1) * NT],
                            start=False,
                            stop=True,
                        )
                    o1 = o1pool.tile([128, NQ], BF16, tag="o1")
                    o2 = o2pool.tile([128, NQ], BF16, tag="o2")
                    nc.vector.tensor_tensor(
                        o1[:], po1[:],
                        x1_sb[q][:, cc * NQ : (cc + 1) * NQ],
                        op=OP.add,
                    )
                    nc.scalar.activation(o2[:], po2[:], AF.Identity)
                    nc.sync.dma_start(
                        out=out1_d[cc * 128 : (cc + 1) * 128, q * NQ : (q + 1) * NQ],
                        in_=o1[:],
                    )
                    nc.sync.dma_start(
                        out=out2_d[cc * 128 : (cc + 1) * 128, q * NQ : (q + 1) * NQ],
                        in_=o2[:],
                    )
                    if nxt is not None:
                        (nxt.stage0, nxt.stage1, nxt.stage2, nxt.stage3)[cc]()
                if nxt is not None:
                    aT = nxt.aT
    nc.compile()
    return nc


def _get_nc():
    if "nc" not in _CACHE:
        _CACHE["nc"] = _build_nc()
    return _CACHE["nc"]


def kernel(x1, y1, x2, y2, wq, bq, wk, bk, wv, bv, scale, scale1, **run_kwargs):
    x1 = np.asarray(x1, np.float32).astype(NP_BF16)
    x2 = np.asarray(x2, np.float32).astype(NP_BF16)
    y1 = np.asarray(y1, np.float32)
    y2 = np.asarray(y2, np.float32)
    vecs = np.stack(
        [
            np.asarray(bq, np.float32).reshape(C4),
            np.asarray(bk, np.float32).reshape(C4),
            np.full(C4, np.asarray(scale).reshape(-1)[0], np.float32),
            np.full(C4, np.asarray(scale1).reshape(-1)[0], np.float32),
        ],
        axis=1,
    )
    rows = np.concatenate(
        [np.asarray(bv, np.float32).reshape(C), np.ones(K, np.float32)]
    ).reshape(1, C + K)
    shared = {
        "wqT": np.ascontiguousarray(np.asarray(wq, np.float32).T).astype(NP_BF16),
        "wkT": np.ascontiguousarray(np.asarray(wk, np.float32).T).astype(NP_BF16),
        "wvT": np.ascontiguousarray(np.asarray(wv, np.float32).T).astype(NP_BF16),
        "vecs": np.ascontiguousarray(vecs),
        "rows": rows.astype(NP_BF16),
    }
    in_maps = []
    for b in range(B):
        in_maps.append(
            {
                "x1": np.ascontiguousarray(x1[b].reshape(C, N)),
                "x2": np.ascontiguousarray(x2[b].reshape(C, N)),
                "y1T": np.ascontiguousarray(y1[b].T).astype(NP_BF16),
                "y2T": np.ascontiguousarray(y2[b].T).astype(NP_BF16),
                **shared,
            }
        )
    nc = _get_nc()
    res = run_bass_kernel_spmd(nc, in_maps, list(range(B)), **run_kwargs)
    _CACHE["last_results"] = res
    out1 = np.stack(
        [res.results[b]["out1"].astype(np.float32).reshape(C, W, H) for b in range(B)]
    )
    out2 = np.stack(
        [res.results[b]["out2"].astype(np.float32).reshape(C, W, H) for b in range(B)]
    )
    return (out1, out2)
